# revision 8
# baseline (speedup 1.0000x reference)
"""Trainium2 8-core kernel for nn_MultiHeadAttention_83408264889124.

Full inputs in, full output out. Sharding: batch (4) x head-group (2) grid
over 8 NeuronCores — each core computes one batch with 6 of the 12 heads and
produces a clean half of the output rows (see scramble note below). All
device work is in transposed layouts so no on-device transposes are needed:

  Qt = (wq/sqrt(K))^T X^T, Kt = wk^T X^T          [K, S] per head
  St[k,q] = sum_d Kt[d,k] Qt[d,q]                  (2-head row-packed matmuls,
                                                    concurrent via PE row tiles)
  E = exp(St)  (ScalarE, PSUM->SBUF bf16)
  AV with stationary [V_h | ones*64]: U[0:64] = V^T E, U[64:128] = colsum(E)
  Ot = U[0:64] * recip(U[64:128])                  (VectorE)

The reference does a RAW reshape [B,H,S,V] -> [B,S,H*V] (no transpose) before
W_o, which scrambles (head, seq): with t = S*h + s, output row s' = t//H gets
feature block j = t%H from head h, position s. Because S*HPC/H = 1024 exactly,
each head-group core produces a clean half of the output rows; the normalize
step writes Ot strided (step H in s) into G^T tiles laid out as rows 64j+v,
and the output projection is Y^T = W_o^T @ G^T with the FULL W_o.

Schedule (the point of this version): the exp stream on ScalarE is the
bottleneck (192 instructions of [128,1024] at ~1.15us each = 211us, 100%
busy).  Everything else is arranged to hide under it:
  - DMA is emitted in critical-path order (wk, xk, wv, xv-chunk0, wq,
    xq-chunk0 first) and only the pair-0 K projection + V chunk0 + Q chunk0
    run before attention starts, so the first exp fires at ~15-20us instead
    of ~64us.
  - All remaining projections are emitted in a LOW-priority band
    (tc.high_priority(offset=-1e6)): the sim-driven Tile scheduler runs them
    only in PE bubbles of the attention phase, never delaying QK^T -> exp.
  - W_o is split into 3 column chunks [0,341),[341,682),[682,1024) that are
    exactly the columns produced by head pairs (0,1),(2,3),(4,5); each chunk
    is emitted right after its pair finishes (lower-priority band), so 2/3
    of the output projection hides inside attention and the tail is ~6us
    (and the PE never idles long enough to go HAM-cold).

mask is all-ones for this problem (spec fill="ones") and adds 0 to logits, so
it is not read. Compute dtype bf16 (inputs converted host-side), f32
accumulation; softmax without max-subtraction (logits are O(1) by
construction so exp never overflows).
"""

from contextlib import ExitStack

import numpy as np
import ml_dtypes

import concourse.bacc as bacc
import concourse.bass as bass
import concourse.mybir as mybir
import concourse.tile as tile
from concourse.bass_utils import run_bass_kernel_spmd

BF16 = mybir.dt.bfloat16
F32 = mybir.dt.float32
I32 = mybir.dt.int32
EXP = mybir.ActivationFunctionType.Exp
RECIP_MAGIC = 0x7EF311C3

B, S, D, H, K, V = 4, 2048, 768, 12, 64, 64
HPC = 6  # heads per core
CH = 512  # q chunk

# W_o column-chunk boundaries: chunk p covers exactly the G^T columns
# produced by head pair p = (2p, 2p+1).  c = t//H with t = S*hl + s; pair p
# covers t in [2048*2p, 2048*(2p+2)) -> c in [ceil(4096p/12)... computed:
WO_BOUNDS = [0, 341, 682, 1024]


def build_nc(S=S, D=D, HPC=HPC, K=K, CH=CH):
    """Build the per-core Bass program (SPMD: same program on all 8 cores)."""
    assert D % 128 == 0 and S % 128 == 0 and S % CH == 0 and K == 64
    DT = D // 128  # contraction tiles for projections
    KT = S // 128  # key-position tiles
    QC = S // CH  # q chunks
    NP = HPC // 2  # head pairs
    FW = HPC * K  # per-core projection feature width (384)
    GT = D // 128
    assert FW // 128 == NP

    nc = bacc.Bacc("TRN2", target_bir_lowering=False, debug=False, num_devices=8)

    # x tensors arrive pre-rearranged host-side as [128, QC, DT, CH] (chunk-
    # major) and weights as [128, n*cols] (SBUF layout) so every DMA reads
    # multi-KB contiguous runs per partition — full HBM bandwidth.
    xq = nc.declare_dram_parameter("xq", [128, QC * DT * CH], BF16, isOutput=False)
    xk = nc.declare_dram_parameter("xk", [128, QC * DT * CH], BF16, isOutput=False)
    xv = nc.declare_dram_parameter("xv", [128, QC * DT * CH], BF16, isOutput=False)
    wq = nc.declare_dram_parameter("wq", [128, DT * FW], BF16, isOutput=False)
    wk = nc.declare_dram_parameter("wk", [128, DT * FW], BF16, isOutput=False)
    wv = nc.declare_dram_parameter("wv", [128, DT * FW], BF16, isOutput=False)
    wo = nc.declare_dram_parameter("wo", [128, GT * D], BF16, isOutput=False)
    SOUT = S * HPC // H  # output rows produced by this core (1024)
    y = nc.declare_dram_parameter("y", [D, SOUT], F32, isOutput=True)

    with tile.TileContext(nc) as tc, ExitStack() as ctx:
        xpool = ctx.enter_context(tc.tile_pool(name="xin", bufs=1))
        wpool = ctx.enter_context(tc.tile_pool(name="w", bufs=1))
        qkpool = ctx.enter_context(tc.tile_pool(name="qk", bufs=1))
        vpool = ctx.enter_context(tc.tile_pool(name="vhat", bufs=1))
        opool = ctx.enter_context(tc.tile_pool(name="ot", bufs=1))
        epool = ctx.enter_context(tc.tile_pool(name="exps", bufs=12))
        rpool = ctx.enter_context(tc.tile_pool(name="rec", bufs=3))
        upsb = ctx.enter_context(tc.tile_pool(name="usb", bufs=2))
        ypool = ctx.enter_context(tc.tile_pool(name="yev", bufs=3))
        # PSUM: psS 2x[128,1024]f32 (4 banks) + u 2x[128,512] (2 banks) +
        # filler-chain accumulators 2x[128,512] (2 banks) = 8 banks.
        pspool = ctx.enter_context(tc.tile_pool(name="ps", bufs=2, space="PSUM"))
        upool = ctx.enter_context(tc.tile_pool(name="us", bufs=2, space="PSUM"))
        fpool = ctx.enter_context(tc.tile_pool(name="fill", bufs=2, space="PSUM"))

        def load_wide(dram):
            """DRAM [128, X] (pre-rearranged) -> SBUF tile [128, X]."""
            t = wpool.tile(
                [128, dram.shape[1]], BF16, tag=dram.name, name=dram.name + "_sb"
            )
            nc.sync.dma_start(t[:], dram[:, :])
            return t

        xq_sb = xpool.tile([128, DT * S], BF16, tag="xq")
        xk_sb = xpool.tile([128, DT * S], BF16, tag="xk")
        xv_sb = xpool.tile([128, DT * S], BF16, tag="xv")

        def load_x_chunk(t, dram, qc):
            t3 = t[:].rearrange("p (n m) -> p n m", m=S)
            d4 = dram[:, :].rearrange("p (q n m) -> p q n m", q=QC, n=DT)
            nc.sync.dma_start(
                t3[:, :, qc * CH : qc * CH + CH],
                d4[:, qc, :, :],
            )

        # ---- input DMA in critical-path order (transfers drain roughly in
        # emission order): the exact bytes the first exp needs, then V chunk0
        # for the first AVs, then the K chunks the kt-loop will consume,
        # then the rest; wo last.
        wk_sb = load_wide(wk)
        load_x_chunk(xk_sb, xk, 0)
        wq_sb = load_wide(wq)
        load_x_chunk(xq_sb, xq, 0)
        wv_sb = load_wide(wv)
        load_x_chunk(xv_sb, xv, 0)
        for qc in range(1, QC):
            load_x_chunk(xk_sb, xk, qc)
            load_x_chunk(xv_sb, xv, qc)
        for qc in range(1, QC):
            load_x_chunk(xq_sb, xq, qc)
        wo_sb = load_wide(wo)

        # ---- engine warm-up: dependency-free matmuls so the HAM clock gate
        # is at 8/8 when the first projections issue, and a dummy exp so the
        # ACT table load (~2.7us) happens before the real exp stream starts.
        wu = wpool.tile([128, 128], BF16, tag="warm", name="warm")
        nc.vector.memset(wu[:], 0.0)
        aw_in = wpool.tile([128, 8], F32, tag="actw", name="actw")
        nc.vector.memset(aw_in[:], 0.0)
        aw_out = wpool.tile([128, 8], BF16, tag="actwo", name="actwo")
        nc.scalar.activation(aw_out[:], aw_in[:], EXP)
        pswu = fpool.tile([128, CH], F32, tag="f", name="pswu")
        for _ in range(48):
            nc.tensor.matmul(pswu[:, 0:128], wu[:], wu[:], start=True, stop=True)

        qt_sb = [
            qkpool.tile([128, S], BF16, tag=f"qt{p}", name=f"qt{p}")
            for p in range(NP)
        ]
        kt_sb = [
            qkpool.tile([128, S], BF16, tag=f"kt{p}", name=f"kt{p}")
            for p in range(NP)
        ]
        # G^T tiles: row 64j+v, col c — g-th tile holds j in {2g, 2g+1}
        gt_sb = [
            opool.tile([128, SOUT], BF16, tag=f"gt{g}", name=f"gt{g}")
            for g in range(GT)
        ]
        # vhat[kt]: [128, HPC*128]; head h occupies cols [128h,128h+128) as
        # [V_h (64) | ones (64)] — the ones columns make the AV matmul also
        # produce sum(exp) replicated across partitions 64..127.
        vhat = [
            vpool.tile([128, HPC * 128], BF16, tag=f"vh{k}", name=f"vh{k}")
            for k in range(KT)
        ]

        def proj_chunk(w_sb, x_sb, dst, hp, qc):
            """One [128, CH] slice of the K or Q projection for pair hp."""
            ps = fpool.tile([128, CH], F32, tag="f", name="fps")
            for dt in range(DT):
                nc.tensor.matmul(
                    ps[:],
                    w_sb[:, dt * FW + hp * 128 : dt * FW + hp * 128 + 128],
                    x_sb[:, dt * S + qc * CH : dt * S + qc * CH + CH],
                    start=(dt == 0),
                    stop=(dt == DT - 1),
                )
            nc.vector.tensor_copy(dst[:, qc * CH : qc * CH + CH], ps[:])

        def v_tile(kt):
            """V projection for ALL heads for one key tile (stationary xv)."""
            pv = fpool.tile([128, CH], F32, tag="f", name="fpv")
            for dt in range(DT):
                nc.tensor.matmul(
                    pv[:, 0:FW],
                    xv_sb[:, dt * S + kt * 128 : dt * S + kt * 128 + 128],
                    wv_sb[:, dt * FW : dt * FW + FW],
                    start=(dt == 0),
                    stop=(dt == DT - 1),
                )
            dst3 = vhat[kt][:].rearrange("p (h m) -> p h m", m=128)
            nc.vector.tensor_copy(
                dst3[:, :, 0:64],
                pv[:, 0:FW].rearrange("p (h m) -> p h m", m=64),
            )
            nc.vector.memset(dst3[:, :, 64:128], 1.0)

        def wo_chunk(p):
            """Output projection for G^T columns [WO_BOUNDS[p], WO_BOUNDS[p+1])."""
            c0, c1 = WO_BOUNDS[p], WO_BOUNDS[p + 1]
            n = c1 - c0
            for dt in range(GT):
                py = fpool.tile([128, CH], F32, tag="f", name="fpy")
                for g in range(GT):
                    nc.tensor.matmul(
                        py[:, 0:n],
                        wo_sb[:, g * D + dt * 128 : g * D + dt * 128 + 128],
                        gt_sb[g][:, c0:c1],
                        start=(g == 0),
                        stop=(g == GT - 1),
                    )
                yt = ypool.tile([128, CH], F32, tag="yev", name="yt")
                nc.vector.tensor_copy(yt[:, 0:n], py[:, 0:n])
                nc.sync.dma_start(
                    y[dt * 128 : dt * 128 + 128, c0:c1],
                    yt[:, 0:n],
                )

        # ---- prologue (normal priority): just enough to start the exp
        # stream: K chunk 0 and Q chunk 0 of pair 0.
        proj_chunk(wk_sb, xk_sb, kt_sb[0], 0, 0)
        proj_chunk(wq_sb, xq_sb, qt_sb[0], 0, 0)

        # ---- everything else is PE filler: the scheduler runs these only
        # when the attention stream leaves the PE idle.  Emission order =
        # deadline order.
        with tc.high_priority(offset=-1_000_000):
            for qc in range(1, QC):
                proj_chunk(wk_sb, xk_sb, kt_sb[0], 0, qc)
            for kt in range(KT):
                v_tile(kt)
            for qc in range(1, QC):
                proj_chunk(wq_sb, xq_sb, qt_sb[0], 0, qc)
            for qc in range(QC):
                proj_chunk(wk_sb, xk_sb, kt_sb[1], 1, qc)
            proj_chunk(wq_sb, xq_sb, qt_sb[1], 1, 0)
            for qc in range(QC):
                proj_chunk(wk_sb, xk_sb, kt_sb[2], 2, qc)
            proj_chunk(wq_sb, xq_sb, qt_sb[2], 2, 0)
            for qc in range(1, QC):
                proj_chunk(wq_sb, xq_sb, qt_sb[1], 1, qc)
            for qc in range(1, QC):
                proj_chunk(wq_sb, xq_sb, qt_sb[2], 2, qc)

        # ---- attention: the ScalarE exp stream is the critical path; QK^T
        # pairs run concurrently in the PE array (row tiles 0/64), AV
        # accumulates over kt in PSUM.
        for hp in range(NP):
            for qc in range(QC):
                u_a = upool.tile([128, CH], F32, tag="u", name="ua")
                u_b = upool.tile([128, CH], F32, tag="u", name="ub")
                for kt in range(KT):
                    psS = pspool.tile([128, 2 * CH], F32, tag="s", name="psS")
                    # row-packed pair: head A rows 0-63, head B rows 64-127
                    nc.tensor.matmul(
                        psS[:, 0:CH],
                        kt_sb[hp][0:64, kt * 128 : kt * 128 + 128],
                        qt_sb[hp][0:64, qc * CH : qc * CH + CH],
                        start=True,
                        stop=True,
                    )
                    nc.tensor.matmul(
                        psS[:, CH : 2 * CH],
                        kt_sb[hp][64:128, kt * 128 : kt * 128 + 128],
                        qt_sb[hp][64:128, qc * CH : qc * CH + CH],
                        start=True,
                        stop=True,
                    )
                    es = epool.tile([128, 2 * CH], BF16, tag="es", name="es")
                    nc.scalar.activation(es[:], psS[:], EXP)
                    nc.tensor.matmul(
                        u_a[:],
                        vhat[kt][:, 256 * hp : 256 * hp + 128],
                        es[:, 0:CH],
                        start=(kt == 0),
                        stop=(kt == KT - 1),
                    )
                    nc.tensor.matmul(
                        u_b[:],
                        vhat[kt][:, 256 * hp + 128 : 256 * hp + 256],
                        es[:, CH : 2 * CH],
                        start=(kt == 0),
                        stop=(kt == KT - 1),
                    )
                # Evacuate each accumulator from PSUM with ONE DVE copy so
                # the u slot frees immediately — the next chunk's AV stream
                # must not wait for the reciprocal + scatter work.  On the
                # very last chunk nothing follows, so read PSUM directly.
                last = hp == NP - 1 and qc == QC - 1
                ev = []
                for u in (u_a, u_b):
                    if last:
                        ev.append(u)
                    else:
                        usb = upsb.tile([128, CH], F32, tag="usb", name="usb")
                        nc.vector.tensor_copy(usb[:], u[:])
                        ev.append(usb)
                for u, hl in ((ev[0], 2 * hp), (ev[1], 2 * hp + 1)):
                    # Newton reciprocal of the replicated exp-sums in rows
                    # 64..127 (magic-constant seed + 2 NR passes).  For the
                    # SBUF copy the chain runs at partitions 64-127 (SBUF
                    # inputs of one op must share their start partition) and
                    # only the final product w lands back at partition 0.
                    den = u[64:128, :]
                    den_i = den.bitcast(I32)
                    if last:
                        r = rpool.tile([64, CH], F32, tag="wre", name="rl")
                        t = rpool.tile([64, CH], F32, tag="wre", name="tl")
                    else:
                        r = rpool.tile([128, CH], F32, tag="rec", name="r")[
                            64:128, :
                        ]
                        t = rpool.tile([128, CH], F32, tag="rec", name="t")[
                            64:128, :
                        ]
                    nc.vector.tensor_scalar(
                        r.bitcast(I32), den_i, RECIP_MAGIC, -1,
                        mybir.AluOpType.subtract, mybir.AluOpType.mult,
                    )
                    nc.vector.tensor_mul(t, den, r)
                    w = rpool.tile([64, CH], F32, tag="wre", name="w")
                    nc.vector.scalar_tensor_tensor(
                        w[:], t, 2.0, r,
                        mybir.AluOpType.subtract, mybir.AluOpType.mult,
                    )

                    # scatter-normalize: Ot[v, s] -> G^T[64j+v, c] with
                    # j=(S*hl+s)%H, c=(S*hl+s)//H; strided in s (step H);
                    # (u * -1) * w == u / l
                    cq0 = qc * CH
                    for j in range(H):
                        s0 = (j - S * hl) % H
                        m0 = max(0, -(-(cq0 - s0) // H))
                        s_st = s0 + H * m0
                        if s_st >= cq0 + CH:
                            continue
                        count = (cq0 + CH - 1 - s_st) // H + 1
                        o = s_st - cq0
                        c_st = (S * hl + s_st) // H
                        sl = slice(o, o + H * (count - 1) + 1, H)
                        nc.vector.scalar_tensor_tensor(
                            gt_sb[j // 2][
                                64 * (j % 2) : 64 * (j % 2) + 64,
                                c_st : c_st + count,
                            ],
                            u[0:64, sl],
                            -1.0,
                            w[:, sl],
                            mybir.AluOpType.mult,
                            mybir.AluOpType.mult,
                        )

            # output projection for the columns this pair completed; low
            # priority so it fills PE bubbles of the next pair's attention
            # (the last pair's chunk is the only work left after attention).
            with tc.high_priority(offset=-2_000_000):
                wo_chunk(hp)

        # Warm-keeper: dependency-light matmuls that become runnable only at
        # the end of the attention stream (gated by the psS slot rotation)
        # and sit in the lowest priority band — they fill the PE-idle window
        # while the final normalize runs on VectorE so the HAM clock gate
        # stays at 8/8 for the last W_o chunk.
        with tc.high_priority(offset=-3_000_000):
            pswk = pspool.tile([128, 2 * CH], F32, tag="s", name="pswk")
            for _ in range(96):
                nc.tensor.matmul(
                    pswk[:, 0:128], wu[:], wu[:], start=True, stop=True,
                )

    nc.compile()
    return nc


_NC_CACHE = None


def _get_nc():
    global _NC_CACHE
    if _NC_CACHE is None:
        _NC_CACHE = build_nc()
    return _NC_CACHE


def _rearr_w(w):
    """[D, ncols] -> [128, DT*ncols] SBUF layout (dt-major along columns)."""
    dt = D // 128
    return w.reshape(dt, 128, -1).transpose(1, 0, 2).reshape(128, -1)


def _rearr_x(xT):
    """[D, S] (x transposed) -> [128, QC*DT*CH] chunk-major layout."""
    dt, qc = D // 128, S // CH
    # [dt, 128, qc, CH] -> [128, qc, dt, CH]
    return (
        xT.reshape(dt, 128, qc, CH).transpose(1, 2, 0, 3).reshape(128, -1)
    )


def _prep_in_maps(queries, keys, values, W_q, W_k, W_v, W_o):
    bf = ml_dtypes.bfloat16
    scale = np.float32(1.0 / np.sqrt(K))
    wo_r = _rearr_w(W_o).astype(bf)  # full W_o: the scramble touches all rows
    in_maps = []
    for core in range(8):
        b, hg = divmod(core, 2)
        h0 = hg * HPC
        wq_c = (W_q[h0 : h0 + HPC] * scale).transpose(1, 0, 2).reshape(D, HPC * K)
        wk_c = W_k[h0 : h0 + HPC].transpose(1, 0, 2).reshape(D, HPC * K)
        wv_c = W_v[h0 : h0 + HPC].transpose(1, 0, 2).reshape(D, HPC * V)
        in_maps.append(
            {
                "xq": _rearr_x(queries[b].T).astype(bf),
                "xk": _rearr_x(keys[b].T).astype(bf),
                "xv": _rearr_x(values[b].T).astype(bf),
                "wq": _rearr_w(wq_c).astype(bf),
                "wk": _rearr_w(wk_c).astype(bf),
                "wv": _rearr_w(wv_c).astype(bf),
                "wo": wo_r,
            }
        )
    return in_maps


def run(inputs, trace=False, **spmd_kwargs):
    """Run on 8 cores; returns (full_output [B,S,D] f32, BassKernelResults)."""
    queries = np.asarray(inputs["queries"], np.float32)
    keys = np.asarray(inputs["keys"], np.float32)
    values = np.asarray(inputs["values"], np.float32)
    W_q = np.asarray(inputs["W_q"], np.float32)
    W_k = np.asarray(inputs["W_k"], np.float32)
    W_v = np.asarray(inputs["W_v"], np.float32)
    W_o = np.asarray(inputs["W_o"], np.float32)

    nc = _get_nc()
    in_maps = _prep_in_maps(queries, keys, values, W_q, W_k, W_v, W_o)
    res = run_bass_kernel_spmd(
        nc, in_maps, core_ids=list(range(8)), trace=trace, **spmd_kwargs
    )
    out = np.empty((B, S, D), np.float32)
    half = S * HPC // H  # 1024 output rows per head-group core
    for b in range(B):
        out[b, 0:half] = res.results[2 * b]["y"].T
        out[b, half : 2 * half] = res.results[2 * b + 1]["y"].T
    return out, res


def kernel(**inputs) -> np.ndarray:
    out, _ = run(inputs, trace=False)
    return out
